# revision 1
# baseline (speedup 1.0000x reference)
"""Trainium2 Bass kernel for CronRootAttention (sparse attention).

Shapes (hardcoded): B=2 H=16 S=4096 D=128, W=64, NB=R=64.
Sharding: fused B*H=32 axis split across 8 cores (4 slices/core).

v3 design ("transposed scores"), per (b,h) slice, per 128-query tile i:
  scores are computed TRANSPOSED: S[key, query] via key-stationary QK
  matmuls, so exp(S) is directly the stationary operand for PV --
  no PE transposes, no SBUF p-copies.

  PSUM S tile [128, 320] layout (partition = key, col = query):
    cols   0:128  A: local keys 128i-64 .. 128i+64   (keys 0:128 for i=0)
    cols 128:192  B: local keys 128i+64 .. 128i+128, queries j>=64 only
    cols 192:320  C: interleaved (strided,relay) keys 0..4i+2

  Masks are ADDITIVE (-1e30) pre-exp, applied with identity-stationary
  matmuls; the i-dependent C boundary (3 partial rows at 4i-1..4i+1)
  uses a sliding-diagonal stationary (identBig[:, 125-4i:253-4i]) so a
  single static 128x128 template lands on the right partitions.

  exp (scale folded) -> p [128,320] bf16; PV: p regions are stationary,
  moving v tiles carry a ones-column so out[:,128] = softmax denominator.
  DVE: reciprocal + per-partition scale -> bf16 out, DMA out.
  Pipeline: PV lags QK by 2 tiles so the PE never waits on the
  activation; input DMA is chunked so compute starts early.
"""

import numpy as np
import ml_dtypes

import concourse.bass as bass
import concourse.bacc as bacc
import concourse.tile as tile
from concourse import mybir
from concourse.bass_utils import run_bass_kernel_spmd

BF16 = ml_dtypes.bfloat16
B, H, S, D = 2, 16, 4096, 128
W = 64
NB = S // W          # 64
R = NB               # 64
NCORES = 8
SLICES = B * H // NCORES   # 4
NT = S // 128        # 32 query tiles per slice
NEG = np.float32(-1e30)
SCALE = 1.0 / np.sqrt(np.float32(D))
DV = D + 1           # v columns + ones column
NVT = S // 128 + 1   # 33 shifted v tiles

_prog_cache = {}


def _build_consts():
    c = np.arange(128)[:, None]   # partition = key index within block
    j = np.arange(128)[None, :]   # col = query index within tile
    # A mask (i>=1): key = 128i-64+c, query m = 128i+j: valid j+1<=c<=j+64
    mA = np.where((c >= j + 1) & (c <= j + 64), 0.0, NEG).astype(np.float32)
    # B mask: key = 128i+64+c (c<64), query j'=j-64 (cols 64): valid c<=j'
    j2 = np.arange(64)[None, :]
    mB = np.where((c < 64) & (c <= j2), 0.0, NEG).astype(np.float32)
    mAdd = np.concatenate([mA, mB], axis=1)            # [128, 192]
    # A0 mask (i=0): key = c, query m = j: valid j-63<=c<=j; cols 128:192
    # stamp -1e30 over the unused B region (has_written is clear there)
    mAdd0 = np.concatenate(
        [np.where((c <= j) & (c >= j - 63), 0.0, NEG),
         np.full((128, 64), NEG)], axis=1).astype(np.float32)
    # C boundary template: row p lands on C row 4i-1+p via identBig diag
    #   p=0 -> rel 2i-1 (valid j>=63), p=1 -> str 2i (valid j>=64),
    #   p=2 -> rel 2i (valid j==127), p>=3 -> untouched rows (0)
    mC = np.zeros((128, 128), np.float32)
    mC[0, :] = np.where(j[0] >= 63, 0.0, NEG)
    mC[1, :] = np.where(j[0] >= 64, 0.0, NEG)
    mC[2, :] = np.where(j[0] >= 127, 0.0, NEG)
    identBig = np.zeros((128, 253), np.float32)
    identBig[np.arange(128), np.arange(128) + 124] = 1.0
    ident = np.eye(128, dtype=np.float32)
    return (mAdd.astype(BF16), mAdd0.astype(BF16), mC.astype(BF16),
            identBig.astype(BF16), ident.astype(BF16))


def build_program():
    if "nc" in _prog_cache:
        return _prog_cache["nc"]
    dt = mybir.dt
    nc = bacc.Bacc("TRN2", target_bir_lowering=False, debug=False)

    qT_d = nc.declare_dram_parameter("qT", [SLICES, 128, S], dt.bfloat16, isOutput=False)
    kT_d = nc.declare_dram_parameter("kT", [SLICES, 128, S], dt.bfloat16, isOutput=False)
    vsh_d = nc.declare_dram_parameter("vsh", [SLICES, 128, NVT * DV], dt.bfloat16, isOutput=False)
    kTsr_d = nc.declare_dram_parameter("kTsr", [SLICES, 128, 128], dt.bfloat16, isOutput=False)
    vsr_d = nc.declare_dram_parameter("vsr", [SLICES, 128, DV], dt.bfloat16, isOutput=False)
    vn0_d = nc.declare_dram_parameter("vn0", [SLICES, 128, DV], dt.bfloat16, isOutput=False)
    ident_d = nc.declare_dram_parameter("ident", [128, 128], dt.bfloat16, isOutput=False)
    identBig_d = nc.declare_dram_parameter("identBig", [128, 253], dt.bfloat16, isOutput=False)
    mAdd_d = nc.declare_dram_parameter("mAdd", [128, 192], dt.bfloat16, isOutput=False)
    mAdd0_d = nc.declare_dram_parameter("mAdd0", [128, 192], dt.bfloat16, isOutput=False)
    mC_d = nc.declare_dram_parameter("mC", [128, 128], dt.bfloat16, isOutput=False)
    out_d = nc.declare_dram_parameter("out", [SLICES, S, D], dt.bfloat16, isOutput=True)

    from contextlib import ExitStack
    with tile.TileContext(nc) as tc, ExitStack() as ctx:
        cpool = ctx.enter_context(tc.tile_pool(name="consts", bufs=1))
        ident = cpool.tile([128, 128], dt.bfloat16, tag="ident")
        nc.sync.dma_start(ident[:], ident_d[:, :])
        identBig = cpool.tile([128, 253], dt.bfloat16, tag="identBig")
        nc.sync.dma_start(identBig[:], identBig_d[:, :])
        mAdd = cpool.tile([128, 192], dt.bfloat16, tag="mAdd")
        nc.sync.dma_start(mAdd[:], mAdd_d[:, :])
        mAdd0 = cpool.tile([128, 192], dt.bfloat16, tag="mAdd0")
        nc.sync.dma_start(mAdd0[:], mAdd0_d[:, :])
        mC = cpool.tile([128, 128], dt.bfloat16, tag="mC")
        nc.sync.dma_start(mC[:], mC_d[:, :])

        spool = ctx.enter_context(tc.tile_pool(name="slice_in", bufs=2))
        pscores = ctx.enter_context(tc.tile_pool(name="pscores", bufs=3, space="PSUM"))
        pout = ctx.enter_context(tc.tile_pool(name="pout", bufs=3, space="PSUM"))
        wpool = ctx.enter_context(tc.tile_pool(name="work", bufs=3))

        state = {}
        cur = {}

        def qk(i):
            # PSUM has_written semantics: the single start=True clears the
            # whole bank's has_written bits; every later matmul in the tile
            # uses start=False -- fresh regions get overwritten, previously
            # written regions accumulate.  The mask matmuls both add the
            # band masks on live regions AND stamp -1e30 over garbage rows.
            n2 = 4 * i + 2
            Sa = pscores.tile([128, 320], dt.float32, tag="scores")
            qTi = cur["qT"][:, 128 * i:128 * (i + 1)]
            if i == 0:
                nc.tensor.matmul(Sa[:, 0:128], cur["kT"][:, 0:128], qTi,
                                 start=True, stop=False, skip_group_check=True)
                nc.tensor.matmul(Sa[0:n2, 192:320], cur["kTsr"][:, 0:n2], qTi,
                                 start=False, stop=False, skip_group_check=True)
                nc.tensor.matmul(Sa[:, 0:192], ident[:], mAdd0[:],
                                 start=False, stop=False, skip_group_check=True)
            else:
                nc.tensor.matmul(Sa[:, 0:128],
                                 cur["kT"][:, 128 * i - 64:128 * i + 64], qTi,
                                 start=True, stop=False, skip_group_check=True)
                nc.tensor.matmul(Sa[0:64, 128:192],
                                 cur["kT"][:, 128 * i + 64:128 * i + 128],
                                 cur["qT"][:, 128 * i + 64:128 * i + 128],
                                 start=False, stop=False, skip_group_check=True)
                nc.tensor.matmul(Sa[0:n2, 192:320], cur["kTsr"][:, 0:n2], qTi,
                                 start=False, stop=False, skip_group_check=True)
                nc.tensor.matmul(Sa[:, 0:192], ident[:], mAdd[:],
                                 start=False, stop=False, skip_group_check=True)
            nc.tensor.matmul(Sa[:, 192:320], identBig[:, 125 - 4 * i:253 - 4 * i],
                             mC[:], start=False, stop=True, skip_group_check=True)
            p_all = wpool.tile([128, 320], dt.bfloat16, tag="p_all")
            nc.scalar.activation(p_all[:, :], Sa[:, :],
                                 mybir.ActivationFunctionType.Exp, scale=float(SCALE))
            state[i] = (p_all, cur["vsh"], cur["vsr"], cur["vn0"], cur["out_s"])

        def pv(i):
            p_all, vsh, vsr, vn0, out_s = state.pop(i)
            n2 = 4 * i + 2
            O = pout.tile([128, DV], dt.float32, tag="outp")
            if i == 0:
                nc.tensor.matmul(O[:], p_all[:, 0:128], vn0[:],
                                 start=True, stop=False, skip_group_check=True)
            else:
                nc.tensor.matmul(O[:], p_all[:, 0:128],
                                 vsh[:, DV * i:DV * (i + 1)],
                                 start=True, stop=False, skip_group_check=True)
                nc.tensor.matmul(O[64:128, :], p_all[0:64, 128:192],
                                 vsh[0:64, DV * (i + 1):DV * (i + 2)],
                                 start=False, stop=False, skip_group_check=True)
            nc.tensor.matmul(O[:], p_all[0:n2, 192:320], vsr[0:n2, :],
                             start=False, stop=True, skip_group_check=True)
            rsum = wpool.tile([128, 1], dt.float32, tag="rsum")
            nc.vector.reciprocal(rsum[:], O[:, 128:129])
            out_sb = wpool.tile([128, 128], dt.bfloat16, tag="out_sb")
            nc.vector.tensor_scalar_mul(out_sb[:], O[:, 0:128], rsum[:])
            nc.sync.dma_start(out_d[out_s, 128 * i:128 * (i + 1), :], out_sb[:])

        CH = 1024            # qT/kT DMA chunk (columns)
        VCH = 9 * DV         # vsh DMA chunk (about a quarter)
        for s in range(SLICES):
            kTsr = spool.tile([128, 128], dt.bfloat16, tag="kTsr")
            nc.sync.dma_start(kTsr[:], kTsr_d[s])
            vsr = spool.tile([128, DV], dt.bfloat16, tag="vsr")
            nc.sync.dma_start(vsr[:], vsr_d[s])
            vn0 = spool.tile([128, DV], dt.bfloat16, tag="vn0")
            nc.sync.dma_start(vn0[:], vn0_d[s])
            qT = spool.tile([128, S], dt.bfloat16, tag="qT")
            kT = spool.tile([128, S], dt.bfloat16, tag="kT")
            vsh = spool.tile([128, NVT * DV], dt.bfloat16, tag="vsh")
            for c0 in range(0, S, CH):
                nc.sync.dma_start(kT[:, c0:c0 + CH], kT_d[s, :, c0:c0 + CH])
                nc.sync.dma_start(qT[:, c0:c0 + CH], qT_d[s, :, c0:c0 + CH])
                v0 = c0 // CH * VCH
                v1 = min(v0 + VCH, NVT * DV)
                nc.sync.dma_start(vsh[:, v0:v1], vsh_d[s, :, v0:v1])
            cur.update(qT=qT, kT=kT, vsh=vsh, kTsr=kTsr, vsr=vsr, vn0=vn0, out_s=s)
            for i in range(NT):
                qk(i)
                if i >= 2:
                    pv(i - 2)
            pv(NT - 2)
            pv(NT - 1)

    nc.finalize()
    _prog_cache["nc"] = nc
    return nc


def _prep_core_inputs(q, k, v, rk, rv, consts):
    """q,k,v: [SLICES, S, D] fp32 for one core; rk, rv: [SLICES, R, D]."""
    mAdd, mAdd0, mC, identBig, ident = consts
    qb = q.astype(BF16)
    kb = k.astype(BF16)
    vb = v.astype(BF16)
    qT = np.ascontiguousarray(qb.transpose(0, 2, 1))          # [SL, 128, S]
    kT = np.ascontiguousarray(kb.transpose(0, 2, 1))
    # 64-shifted padded v tiles augmented with a ones column, stored
    # per-partition-contiguous: [SL, 128, NVT*DV]; tile j = v rows 128j-64..128j+64
    vpad = np.concatenate([np.zeros((SLICES, 64, D), BF16), vb,
                           np.zeros((SLICES, 64, D), BF16)], axis=1)  # [SL, 4224, D]
    vpad = np.concatenate([vpad, np.ones((SLICES, NVT * 128, 1), BF16)], axis=2)
    vsh = np.ascontiguousarray(
        vpad.reshape(SLICES, NVT, 128, DV).transpose(0, 2, 1, 3).reshape(SLICES, 128, NVT * DV))
    # interleaved strided/relay keys, d-major: col 2s = k[64s], col 2s+1 = rk[s]
    ksr_int = np.empty((SLICES, 128, D), BF16)
    ksr_int[:, 0::2] = kb[:, ::W, :]
    ksr_int[:, 1::2] = rk.astype(BF16)
    kTsr = np.ascontiguousarray(ksr_int.transpose(0, 2, 1))           # [SL, 128, 128]
    # interleaved [str0, rel0, str1, rel1, ...] + ones column
    vsr_pairs = np.empty((SLICES, 128, D), BF16)
    vsr_pairs[:, 0::2] = vb[:, ::W, :]
    vsr_pairs[:, 1::2] = rv.astype(BF16)
    vsr = np.ascontiguousarray(
        np.concatenate([vsr_pairs, np.ones((SLICES, 128, 1), BF16)], axis=2))
    vn0 = np.ascontiguousarray(
        np.concatenate([vb[:, 0:128, :], np.ones((SLICES, 128, 1), BF16)], axis=2))
    return {
        "qT": qT, "kT": kT, "vsh": vsh, "kTsr": kTsr, "vsr": vsr, "vn0": vn0,
        "ident": ident, "identBig": identBig, "mAdd": mAdd, "mAdd0": mAdd0,
        "mC": mC,
    }


def make_in_maps(q, k, v, rk, rv):
    consts = _build_consts()
    qf = q.reshape(B * H, S, D)
    kf = k.reshape(B * H, S, D)
    vf = v.reshape(B * H, S, D)
    rkf = rk.reshape(B * H, R, D)
    rvf = rv.reshape(B * H, R, D)
    in_maps = []
    for c in range(NCORES):
        sl = slice(SLICES * c, SLICES * (c + 1))
        in_maps.append(_prep_core_inputs(qf[sl], kf[sl], vf[sl], rkf[sl], rvf[sl],
                                         consts))
    return in_maps


def kernel(q, k, v, rk, rv, _run_kwargs=None):
    q = np.asarray(q, dtype=np.float32)
    k = np.asarray(k, dtype=np.float32)
    v = np.asarray(v, dtype=np.float32)
    rk = np.asarray(rk, dtype=np.float32)
    rv = np.asarray(rv, dtype=np.float32)
    nc = build_program()
    in_maps = make_in_maps(q, k, v, rk, rv)
    res = run_bass_kernel_spmd(nc, in_maps, list(range(NCORES)), **(_run_kwargs or {}))
    out = np.stack([np.asarray(res.results[c]["out"]) for c in range(NCORES)])
    if _run_kwargs:
        kernel.last_results = res
    return out.reshape(B, H, S, D).astype(np.float32)



# revision 2
# speedup vs baseline: 1.4164x; 1.4164x over previous
"""Trainium2 Bass kernel for CronRootAttention (sparse attention).

Shapes (hardcoded): B=2 H=16 S=4096 D=128, W=64, NB=R=64.
Sharding: fused B*H=32 axis split across 8 cores (4 slices/core).

v4 design ("transposed scores + multiplicative masks"), per (b,h) slice,
per 128-query tile i (group g = i//4 covers 4 tiles = 512 queries):

  scores computed TRANSPOSED: S[key, query] via key-stationary QK
  matmuls so exp(S) is directly the stationary operand for PV.

  Local scores per tile in PSUM Sa [128, 192]:
    cols   0:128  A: keys 128i-64 .. 128i+64   (keys 0:128 for i=0)
    cols 128:192  B: keys 128i+64 .. 128i+128, queries j>=64 (skip i=0)
  Strided/relay scores per GROUP in PSUM Sc [128, 512]:
    one matmul: stationary kTsr (all 128 interleaved strided/relay keys,
    loaded once per group), moving qT[:, 512g:512g+512].

  NO additive -1e30 mask matmuls on the PE.  Instead exp() runs on the
  raw scores (stale PSUM regions are bounded, see memsets below) and the
  band/causal masks are applied POST-exp as multiplicative 0/1 bf16
  masks: AB mask on GpSimd (idle engine), C mask on Vector.  Zeroed p
  rows contribute nothing to PV numerator or the ones-column denominator.

  PV per tile: p regions stationary, moving v tiles carry a ones-column
  so O[:,128] = softmax denominator; DVE reciprocal + per-partition
  scale writes a bf16 out column block; ONE output DMA per slice
  (contiguous [128, 32*128] SBUF -> [128, NT*128] DRAM, 8KB lines).

  PSUM banks: Sa x3 + Sc x2 + O x3 = 8.  Sa banks are memset once at
  start so the first exp of never-written regions can't see +huge
  garbage (exp -> inf -> 0*inf = NaN); afterwards stale data is always
  old scores (|s| <= ~70 -> exp(s*scale) <= ~500, finite, then x0).

  Pipeline: PV lags QK by 2 tiles; the group matmul Sc is emitted ahead
  of the group's A/B matmuls so its long 512-col stream hides their
  weight loads; input DMA is chunked so compute starts early.
"""

import numpy as np
import ml_dtypes

import concourse.bass as bass
import concourse.bacc as bacc
import concourse.tile as tile
from concourse import mybir
from concourse.bass_utils import run_bass_kernel_spmd

BF16 = ml_dtypes.bfloat16
B, H, S, D = 2, 16, 4096, 128
W = 64
NB = S // W          # 64
R = NB               # 64
NCORES = 8
SLICES = B * H // NCORES   # 4
NT = S // 128        # 32 query tiles per slice
GT = 4               # tiles per strided-score group
NG = NT // GT        # 8 groups per slice
SCALE = 1.0 / np.sqrt(np.float32(D))
DV = D + 1           # v columns + ones column
NVT = S // 128 + 1   # 33 shifted v tiles

_prog_cache = {}


def _build_consts():
    c = np.arange(128)[:, None]   # partition = key index within region
    j = np.arange(128)[None, :]   # col = query index within tile
    # AB mask (i>=1), multiplicative:
    #  A cols 0:128: key = 128i-64+c, query m = 128i+j: valid j+1<=c<=j+64
    #  B cols 128:192: key = 128i+64+c (c<64), query j'=j-64: valid c<=j'
    mA = ((c >= j + 1) & (c <= j + 64)).astype(np.float32)
    j2 = np.arange(64)[None, :]
    mB = ((c < 64) & (c <= j2)).astype(np.float32)
    mAB = np.concatenate([mA, mB], axis=1)             # [128, 192]
    # i=0 variant: key = c, query m = j: valid j-63<=c<=j; B region zero
    mAB0 = np.concatenate(
        [((c <= j) & (c >= j - 63)).astype(np.float32),
         np.zeros((128, 64), np.float32)], axis=1)
    # C masks per group g: [128, 512]; row 2s = strided key s (pos 64s),
    # row 2s+1 = relay s (block end 64s+63); query m = 512g + q.
    # valid strided: 64s < max(m-63,0); valid relay: 64s+63 < max(m-63,0)
    mC = np.zeros((NG, 128, 512), np.float32)
    s_ = np.arange(64)[:, None]
    for g in range(NG):
        m = (512 * g + np.arange(512))[None, :]
        ls = np.maximum(m - 63, 0)
        mC[g, 0::2, :] = (64 * s_ < ls).astype(np.float32)
        mC[g, 1::2, :] = (64 * s_ + 63 < ls).astype(np.float32)
    mCg = mC.transpose(1, 0, 2).reshape(128, NG * 512)  # [128, 8*512]
    return mAB.astype(BF16), mAB0.astype(BF16), np.ascontiguousarray(mCg).astype(BF16)


def build_program():
    if "nc" in _prog_cache:
        return _prog_cache["nc"]
    dt = mybir.dt
    nc = bacc.Bacc("TRN2", target_bir_lowering=False, debug=False)

    qT_d = nc.declare_dram_parameter("qT", [SLICES, 128, S], dt.bfloat16, isOutput=False)
    kT_d = nc.declare_dram_parameter("kT", [SLICES, 128, S], dt.bfloat16, isOutput=False)
    vsh_d = nc.declare_dram_parameter("vsh", [SLICES, 128, NVT * DV], dt.bfloat16, isOutput=False)
    kTsr_d = nc.declare_dram_parameter("kTsr", [SLICES, 128, 128], dt.bfloat16, isOutput=False)
    vsr_d = nc.declare_dram_parameter("vsr", [SLICES, 128, DV], dt.bfloat16, isOutput=False)
    vn0_d = nc.declare_dram_parameter("vn0", [SLICES, 128, DV], dt.bfloat16, isOutput=False)
    mAB_d = nc.declare_dram_parameter("mAB", [128, 192], dt.bfloat16, isOutput=False)
    mAB0_d = nc.declare_dram_parameter("mAB0", [128, 192], dt.bfloat16, isOutput=False)
    mCg_d = nc.declare_dram_parameter("mCg", [128, NG * 512], dt.bfloat16, isOutput=False)
    # out stored [slice, partition(=query%128), tile*128+d]; host transposes
    out_d = nc.declare_dram_parameter("out", [SLICES, 128, NT * D], dt.bfloat16, isOutput=True)

    from contextlib import ExitStack
    with tile.TileContext(nc) as tc, ExitStack() as ctx:
        cpool = ctx.enter_context(tc.tile_pool(name="consts", bufs=1))
        mAB = cpool.tile([128, 192], dt.bfloat16, tag="mAB")
        nc.sync.dma_start(mAB[:], mAB_d[:, :])
        mAB0 = cpool.tile([128, 192], dt.bfloat16, tag="mAB0")
        nc.sync.dma_start(mAB0[:], mAB0_d[:, :])
        mCg = cpool.tile([128, NG * 512], dt.bfloat16, tag="mCg")
        nc.sync.dma_start(mCg[:], mCg_d[:, :])

        spool = ctx.enter_context(tc.tile_pool(name="slice_in", bufs=2))
        pscores = ctx.enter_context(tc.tile_pool(name="pscores", bufs=3, space="PSUM"))
        pcpool = ctx.enter_context(tc.tile_pool(name="pcscores", bufs=2, space="PSUM"))
        pout = ctx.enter_context(tc.tile_pool(name="pout", bufs=3, space="PSUM"))
        wpool = ctx.enter_context(tc.tile_pool(name="work", bufs=3))
        gpool = ctx.enter_context(tc.tile_pool(name="gwork", bufs=2))
        opool = ctx.enter_context(tc.tile_pool(name="outacc", bufs=2))

        # one-time: clear the Sa banks so first-use stale PSUM can't be huge
        for z in range(3):
            zt = pscores.tile([128, 192], dt.float32, tag="scores")
            nc.vector.memset(zt[:], 0.0)

        state = {}
        gstate = {}
        cur = {}

        def cgroup(g):
            Sc = pcpool.tile([128, 512], dt.float32, tag="cscores")
            nc.tensor.matmul(Sc[:, :], cur["kTsr"][:, 0:128],
                             cur["qT"][:, 512 * g:512 * (g + 1)],
                             start=True, stop=True, skip_group_check=True)
            pc = gpool.tile([128, 512], dt.bfloat16, tag="pc")
            nc.scalar.activation(pc[:, :], Sc[:, :],
                                 mybir.ActivationFunctionType.Exp, scale=float(SCALE))
            pcm = gpool.tile([128, 512], dt.bfloat16, tag="pcm")
            nc.vector.tensor_tensor(pcm[:, :], pc[:, :],
                                    mCg[:, 512 * g:512 * (g + 1)],
                                    mybir.AluOpType.mult)
            gstate[g] = pcm

        def qk(i):
            Sa = pscores.tile([128, 192], dt.float32, tag="scores")
            qTi = cur["qT"][:, 128 * i:128 * (i + 1)]
            if i == 0:
                nc.tensor.matmul(Sa[:, 0:128], cur["kT"][:, 0:128], qTi,
                                 start=True, stop=True, skip_group_check=True)
            else:
                nc.tensor.matmul(Sa[:, 0:128],
                                 cur["kT"][:, 128 * i - 64:128 * i + 64], qTi,
                                 start=True, stop=False, skip_group_check=True)
                nc.tensor.matmul(Sa[0:64, 128:192],
                                 cur["kT"][:, 128 * i + 64:128 * i + 128],
                                 cur["qT"][:, 128 * i + 64:128 * i + 128],
                                 start=False, stop=True, skip_group_check=True)
            p_ab = wpool.tile([128, 192], dt.bfloat16, tag="p_ab")
            nc.scalar.activation(p_ab[:, :], Sa[:, :],
                                 mybir.ActivationFunctionType.Exp, scale=float(SCALE))
            p_abm = wpool.tile([128, 192], dt.bfloat16, tag="p_abm")
            nc.gpsimd.tensor_tensor(p_abm[:, :], p_ab[:, :],
                                    (mAB0 if i == 0 else mAB)[:, :],
                                    mybir.AluOpType.mult)
            state[i] = (p_abm, cur["vsh"], cur["vsr"], cur["vn0"], cur["out_acc"],
                        gstate[i // GT])

        def pv(i):
            p_abm, vsh, vsr, vn0, out_acc, pcm = state.pop(i)
            t = i % GT
            O = pout.tile([128, DV], dt.float32, tag="outp")
            if i == 0:
                nc.tensor.matmul(O[:], p_abm[:, 0:128], vn0[:],
                                 start=True, stop=False, skip_group_check=True)
            else:
                nc.tensor.matmul(O[:], p_abm[:, 0:128],
                                 vsh[:, DV * i:DV * (i + 1)],
                                 start=True, stop=False, skip_group_check=True)
                nc.tensor.matmul(O[64:128, :], p_abm[0:64, 128:192],
                                 vsh[0:64, DV * (i + 1):DV * (i + 2)],
                                 start=False, stop=False, skip_group_check=True)
            nc.tensor.matmul(O[:], pcm[:, 128 * t:128 * (t + 1)], vsr[:],
                             start=False, stop=True, skip_group_check=True)
            rsum = wpool.tile([128, 1], dt.float32, tag="rsum")
            nc.vector.reciprocal(rsum[:], O[:, 128:129])
            nc.vector.tensor_scalar_mul(out_acc[:, 128 * i:128 * (i + 1)],
                                        O[:, 0:128], rsum[:])

        CH = 1024            # qT/kT DMA chunk (columns)
        VCH = 9 * DV         # vsh DMA chunk (about a quarter)
        for s in range(SLICES):
            kTsr = spool.tile([128, 128], dt.bfloat16, tag="kTsr")
            nc.sync.dma_start(kTsr[:], kTsr_d[s])
            vsr = spool.tile([128, DV], dt.bfloat16, tag="vsr")
            nc.sync.dma_start(vsr[:], vsr_d[s])
            vn0 = spool.tile([128, DV], dt.bfloat16, tag="vn0")
            nc.sync.dma_start(vn0[:], vn0_d[s])
            qT = spool.tile([128, S], dt.bfloat16, tag="qT")
            kT = spool.tile([128, S], dt.bfloat16, tag="kT")
            vsh = spool.tile([128, NVT * DV], dt.bfloat16, tag="vsh")
            for c0 in range(0, S, CH):
                nc.sync.dma_start(kT[:, c0:c0 + CH], kT_d[s, :, c0:c0 + CH])
                nc.sync.dma_start(qT[:, c0:c0 + CH], qT_d[s, :, c0:c0 + CH])
                v0 = c0 // CH * VCH
                v1 = min(v0 + VCH, NVT * DV)
                nc.sync.dma_start(vsh[:, v0:v1], vsh_d[s, :, v0:v1])
            out_acc = opool.tile([128, NT * D], dt.bfloat16, tag="out_acc")
            cur.update(qT=qT, kT=kT, vsh=vsh, kTsr=kTsr, vsr=vsr, vn0=vn0,
                       out_acc=out_acc)
            for i in range(NT):
                if i % GT == 0:
                    cgroup(i // GT)
                qk(i)
                if i >= 2:
                    pv(i - 2)
            pv(NT - 2)
            pv(NT - 1)
            nc.sync.dma_start(out_d[s], out_acc[:])

    nc.finalize()
    _prog_cache["nc"] = nc
    return nc


def _prep_core_inputs(q, k, v, rk, rv, consts):
    """q,k,v: [SLICES, S, D] fp32 for one core; rk, rv: [SLICES, R, D]."""
    mAB, mAB0, mCg = consts
    qb = q.astype(BF16)
    kb = k.astype(BF16)
    vb = v.astype(BF16)
    qT = np.ascontiguousarray(qb.transpose(0, 2, 1))          # [SL, 128, S]
    kT = np.ascontiguousarray(kb.transpose(0, 2, 1))
    # 64-shifted padded v tiles augmented with a ones column, stored
    # per-partition-contiguous: [SL, 128, NVT*DV]; tile j = v rows 128j-64..128j+64
    vpad = np.concatenate([np.zeros((SLICES, 64, D), BF16), vb,
                           np.zeros((SLICES, 64, D), BF16)], axis=1)  # [SL, 4224, D]
    vpad = np.concatenate([vpad, np.ones((SLICES, NVT * 128, 1), BF16)], axis=2)
    vsh = np.ascontiguousarray(
        vpad.reshape(SLICES, NVT, 128, DV).transpose(0, 2, 1, 3).reshape(SLICES, 128, NVT * DV))
    # interleaved strided/relay keys, d-major: col 2s = k[64s], col 2s+1 = rk[s]
    ksr_int = np.empty((SLICES, 128, D), BF16)
    ksr_int[:, 0::2] = kb[:, ::W, :]
    ksr_int[:, 1::2] = rk.astype(BF16)
    kTsr = np.ascontiguousarray(ksr_int.transpose(0, 2, 1))           # [SL, 128, 128]
    # interleaved [str0, rel0, str1, rel1, ...] + ones column
    vsr_pairs = np.empty((SLICES, 128, D), BF16)
    vsr_pairs[:, 0::2] = vb[:, ::W, :]
    vsr_pairs[:, 1::2] = rv.astype(BF16)
    vsr = np.ascontiguousarray(
        np.concatenate([vsr_pairs, np.ones((SLICES, 128, 1), BF16)], axis=2))
    vn0 = np.ascontiguousarray(
        np.concatenate([vb[:, 0:128, :], np.ones((SLICES, 128, 1), BF16)], axis=2))
    return {
        "qT": qT, "kT": kT, "vsh": vsh, "kTsr": kTsr, "vsr": vsr, "vn0": vn0,
        "mAB": mAB, "mAB0": mAB0, "mCg": mCg,
    }


def make_in_maps(q, k, v, rk, rv):
    consts = _build_consts()
    qf = q.reshape(B * H, S, D)
    kf = k.reshape(B * H, S, D)
    vf = v.reshape(B * H, S, D)
    rkf = rk.reshape(B * H, R, D)
    rvf = rv.reshape(B * H, R, D)
    in_maps = []
    for c in range(NCORES):
        sl = slice(SLICES * c, SLICES * (c + 1))
        in_maps.append(_prep_core_inputs(qf[sl], kf[sl], vf[sl], rkf[sl], rvf[sl],
                                         consts))
    return in_maps


def kernel(q, k, v, rk, rv, _run_kwargs=None):
    q = np.asarray(q, dtype=np.float32)
    k = np.asarray(k, dtype=np.float32)
    v = np.asarray(v, dtype=np.float32)
    rk = np.asarray(rk, dtype=np.float32)
    rv = np.asarray(rv, dtype=np.float32)
    nc = build_program()
    in_maps = make_in_maps(q, k, v, rk, rv)
    res = run_bass_kernel_spmd(nc, in_maps, list(range(NCORES)), **(_run_kwargs or {}))
    out = np.stack([np.asarray(res.results[c]["out"]) for c in range(NCORES)])
    if _run_kwargs:
        kernel.last_results = res
    # out: [NCORES, SLICES, 128, NT*D] -> [B,H,S,D]
    out = out.reshape(B * H, 128, NT, D).transpose(0, 2, 1, 3)
    return out.reshape(B, H, S, D).astype(np.float32)


# revision 5
# speedup vs baseline: 1.7135x; 1.2097x over previous
"""Trainium2 Bass kernel for CronRootAttention (sparse attention).

Shapes (hardcoded): B=2 H=16 S=4096 D=128, W=64, NB=R=64.
Sharding: fused B*H=32 axis split across 8 cores (4 slices/core).

v4 design ("transposed scores + multiplicative masks"), per (b,h) slice,
per 128-query tile i (group g = i//4 covers 4 tiles = 512 queries):

  scores computed TRANSPOSED: S[key, query] via key-stationary QK
  matmuls so exp(S) is directly the stationary operand for PV.

  Local scores per tile in PSUM Sa [128, 192]:
    cols   0:128  A: keys 128i-64 .. 128i+64   (keys 0:128 for i=0)
    cols 128:192  B: keys 128i+64 .. 128i+128, queries j>=64 (skip i=0)
  Strided/relay scores per GROUP in PSUM Sc [128, 512]:
    one matmul: stationary kTsr (all 128 interleaved strided/relay keys,
    loaded once per group), moving qT[:, 512g:512g+512].

  NO additive -1e30 mask matmuls on the PE.  Instead exp() runs on the
  raw scores (stale PSUM regions are bounded, see memsets below) and the
  band/causal masks are applied POST-exp as multiplicative 0/1 bf16
  masks: AB mask on GpSimd (idle engine), C mask on Vector.  Zeroed p
  rows contribute nothing to PV numerator or the ones-column denominator.

  PV per tile: p regions stationary, moving v tiles carry a ones-column
  so O[:,128] = softmax denominator; DVE reciprocal + per-partition
  scale writes a bf16 out column block; ONE output DMA per slice
  (contiguous [128, 32*128] SBUF -> [128, NT*128] DRAM, 8KB lines).

  PSUM banks: Sa x3 + Sc x2 + O x3 = 8.  Sa banks are memset once at
  start so the first exp of never-written regions can't see +huge
  garbage (exp -> inf -> 0*inf = NaN); afterwards stale data is always
  old scores (|s| <= ~70 -> exp(s*scale) <= ~500, finite, then x0).

  Pipeline: PV lags QK by 2 tiles; the group matmul Sc is emitted ahead
  of the group's A/B matmuls so its long 512-col stream hides their
  weight loads; input DMA is chunked so compute starts early.
"""

import numpy as np
import ml_dtypes

import concourse.bass as bass
import concourse.bacc as bacc
import concourse.tile as tile
from concourse import mybir
from concourse.bass_utils import run_bass_kernel_spmd

BF16 = ml_dtypes.bfloat16
B, H, S, D = 2, 16, 4096, 128
W = 64
NB = S // W          # 64
R = NB               # 64
NCORES = 8
SLICES = B * H // NCORES   # 4
NT = S // 128        # 32 query tiles per slice
GT = 4               # tiles per strided-score group
NG = NT // GT        # 8 groups per slice
SCALE = 1.0 / np.sqrt(np.float32(D))
DV = D + 1           # v columns + ones column
NVT = S // 128 + 1   # 33 shifted v tiles

_prog_cache = {}


def _build_consts():
    c = np.arange(128)[:, None]   # partition = key index within region
    j = np.arange(128)[None, :]   # col = query index within tile
    # AB mask (i>=1), multiplicative:
    #  A cols 0:128: key = 128i-64+c, query m = 128i+j: valid j+1<=c<=j+64
    #  B cols 128:192: key = 128i+64+c (c<64), query j'=j-64: valid c<=j'
    mA = ((c >= j + 1) & (c <= j + 64)).astype(np.float32)
    j2 = np.arange(64)[None, :]
    mB = ((c < 64) & (c <= j2)).astype(np.float32)
    mAB = np.concatenate([mA, mB], axis=1)             # [128, 192]
    # i=0 variant: key = c, query m = j: valid j-63<=c<=j; B region zero
    mAB0 = np.concatenate(
        [((c <= j) & (c >= j - 63)).astype(np.float32),
         np.zeros((128, 64), np.float32)], axis=1)
    # C masks per group g: [128, 512]; row 2s = strided key s (pos 64s),
    # row 2s+1 = relay s (block end 64s+63); query m = 512g + q.
    # valid strided: 64s < max(m-63,0); valid relay: 64s+63 < max(m-63,0)
    mC = np.zeros((NG, 128, 512), np.float32)
    s_ = np.arange(64)[:, None]
    for g in range(NG):
        m = (512 * g + np.arange(512))[None, :]
        ls = np.maximum(m - 63, 0)
        mC[g, 0::2, :] = (64 * s_ < ls).astype(np.float32)
        mC[g, 1::2, :] = (64 * s_ + 63 < ls).astype(np.float32)
    mCg = mC.transpose(1, 0, 2).reshape(128, NG * 512)  # [128, 8*512]
    return mAB.astype(BF16), mAB0.astype(BF16), np.ascontiguousarray(mCg).astype(BF16)


def build_program():
    if "nc" in _prog_cache:
        return _prog_cache["nc"]
    dt = mybir.dt
    nc = bacc.Bacc("TRN2", target_bir_lowering=False, debug=False)

    qT_d = nc.declare_dram_parameter("qT", [SLICES, 128, S], dt.bfloat16, isOutput=False)
    kT_d = nc.declare_dram_parameter("kT", [SLICES, 128, S], dt.bfloat16, isOutput=False)
    vsh_d = nc.declare_dram_parameter("vsh", [SLICES, 128, NVT * DV], dt.bfloat16, isOutput=False)
    kTsr_d = nc.declare_dram_parameter("kTsr", [SLICES, 128, 128], dt.bfloat16, isOutput=False)
    vsr_d = nc.declare_dram_parameter("vsr", [SLICES, 128, DV], dt.bfloat16, isOutput=False)
    vn0_d = nc.declare_dram_parameter("vn0", [SLICES, 128, DV], dt.bfloat16, isOutput=False)
    mAB_d = nc.declare_dram_parameter("mAB", [128, 192], dt.bfloat16, isOutput=False)
    mAB0_d = nc.declare_dram_parameter("mAB0", [128, 192], dt.bfloat16, isOutput=False)
    mCg_d = nc.declare_dram_parameter("mCg", [128, NG * 512], dt.bfloat16, isOutput=False)
    # out stored [slice, partition(=query%128), tile*128+d]; host transposes
    out_d = nc.declare_dram_parameter("out", [SLICES, 128, NT * D], dt.bfloat16, isOutput=True)

    from contextlib import ExitStack
    with tile.TileContext(nc) as tc, ExitStack() as ctx:
        cpool = ctx.enter_context(tc.tile_pool(name="consts", bufs=1))
        mAB = cpool.tile([128, 192], dt.bfloat16, tag="mAB")
        nc.sync.dma_start(mAB[:], mAB_d[:, :])
        mAB0 = cpool.tile([128, 192], dt.bfloat16, tag="mAB0")
        nc.sync.dma_start(mAB0[:], mAB0_d[:, :])
        mCg = cpool.tile([128, NG * 512], dt.bfloat16, tag="mCg")
        # mCg DMA is issued inside slice 0's prologue (split, critical first)

        spool = ctx.enter_context(tc.tile_pool(name="slice_in", bufs=2))
        pscores = ctx.enter_context(tc.tile_pool(name="pscores", bufs=3, space="PSUM"))
        pcpool = ctx.enter_context(tc.tile_pool(name="pcscores", bufs=2, space="PSUM"))
        pout = ctx.enter_context(tc.tile_pool(name="pout", bufs=3, space="PSUM"))
        wpool = ctx.enter_context(tc.tile_pool(name="work", bufs=3))
        gpool = ctx.enter_context(tc.tile_pool(name="gwork", bufs=2))
        opool = ctx.enter_context(tc.tile_pool(name="outacc", bufs=2))

        # one-time: clear the Sa banks so first-use stale PSUM can't be huge
        for z in range(3):
            zt = pscores.tile([128, 192], dt.float32, tag="scores")
            nc.vector.memset(zt[:], 0.0)

        state = {}
        gstate = {}
        cur = {}

        def cgroup(g):
            Sc = pcpool.tile([128, 512], dt.float32, tag="cscores")
            nc.tensor.matmul(Sc[:, :], cur["kTsr"][:, 0:128],
                             cur["qT"][:, 512 * g:512 * (g + 1)],
                             start=True, stop=True, skip_group_check=True)
            pc = gpool.tile([128, 512], dt.bfloat16, tag="pc")
            nc.scalar.activation(pc[:, :], Sc[:, :],
                                 mybir.ActivationFunctionType.Exp, scale=float(SCALE))
            pcm = gpool.tile([128, 512], dt.bfloat16, tag="pcm")
            nc.vector.tensor_tensor(pcm[:, :], pc[:, :],
                                    mCg[:, 512 * g:512 * (g + 1)],
                                    mybir.AluOpType.mult)
            gstate[g] = pcm

        sa_tiles = {}

        def emit_a(i):
            # A scores for tile i.  i>=1 shares its stationary
            # kT[:, 128i-64:128i+64] with the preceding emit_b(i-1) call so
            # walrus can skip the reload (identical weights AP back-to-back).
            Sa = pscores.tile([128, 192], dt.float32, tag="scores")
            sa_tiles[i] = Sa
            qTi = cur["qT"][:, 128 * i:128 * (i + 1)]
            if i == 0:
                nc.tensor.matmul(Sa[:, 0:128], cur["kT"][:, 0:128], qTi,
                                 start=True, stop=True, skip_group_check=True)
            else:
                nc.tensor.matmul(Sa[:, 0:128],
                                 cur["kT"][:, 128 * i - 64:128 * i + 64], qTi,
                                 start=True, stop=True, skip_group_check=True)

        def emit_b(i):
            # B scores for tile i: keys 128i+64..128i+128, queries j>=64.
            # For i<NT-1 use the full 128-col stationary of emit_a(i+1); the
            # extra output rows 64:128 are garbage and masked to 0 post-exp.
            Sa = sa_tiles[i]
            if i < NT - 1:
                nc.tensor.matmul(Sa[:, 128:192],
                                 cur["kT"][:, 128 * i + 64:128 * i + 192],
                                 cur["qT"][:, 128 * i + 64:128 * i + 128],
                                 start=False, stop=True, skip_group_check=True)
            else:
                nc.tensor.matmul(Sa[0:64, 128:192],
                                 cur["kT"][:, 128 * i + 64:128 * i + 128],
                                 cur["qT"][:, 128 * i + 64:128 * i + 128],
                                 start=False, stop=True, skip_group_check=True)

        def finish(i):
            Sa = sa_tiles.pop(i)
            p_ab = wpool.tile([128, 192], dt.bfloat16, tag="p_ab")
            nc.scalar.activation(p_ab[:, :], Sa[:, :],
                                 mybir.ActivationFunctionType.Exp, scale=float(SCALE))
            p_abm = wpool.tile([128, 192], dt.bfloat16, tag="p_abm")
            nc.gpsimd.tensor_tensor(p_abm[:, :], p_ab[:, :],
                                    (mAB0 if i == 0 else mAB)[:, :],
                                    mybir.AluOpType.mult)
            state[i] = (p_abm, cur["vsh"], cur["vsr"], cur["vn0"], cur["out_acc"],
                        gstate[i // GT])

        def pv(i):
            p_abm, vsh, vsr, vn0, out_acc, pcm = state.pop(i)
            t = i % GT
            O = pout.tile([128, DV], dt.float32, tag="outp")
            if i == 0:
                nc.tensor.matmul(O[:], p_abm[:, 0:128], vn0[:],
                                 start=True, stop=False, skip_group_check=True)
            else:
                nc.tensor.matmul(O[:], p_abm[:, 0:128],
                                 vsh[:, DV * i:DV * (i + 1)],
                                 start=True, stop=False, skip_group_check=True)
                nc.tensor.matmul(O[64:128, :], p_abm[0:64, 128:192],
                                 vsh[0:64, DV * (i + 1):DV * (i + 2)],
                                 start=False, stop=False, skip_group_check=True)
            nc.tensor.matmul(O[:], pcm[:, 128 * t:128 * (t + 1)], vsr[:],
                             start=False, stop=True, skip_group_check=True)
            rsum = wpool.tile([128, 1], dt.float32, tag="rsum")
            nc.vector.reciprocal(rsum[:], O[:, 128:129])
            nc.vector.tensor_scalar_mul(out_acc[:, 128 * i:128 * (i + 1)],
                                        O[:, 0:128], rsum[:])

        CH = 1024            # qT/kT DMA chunk (columns)
        VCH = 9 * DV         # vsh DMA chunk (about a quarter)
        OCH = 8 * D          # out DMA chunk (8 tiles)
        for s in range(SLICES):
            kTsr = spool.tile([128, 128], dt.bfloat16, tag="kTsr")
            nc.sync.dma_start(kTsr[:], kTsr_d[s])
            qT = spool.tile([128, S], dt.bfloat16, tag="qT")
            kT = spool.tile([128, S], dt.bfloat16, tag="kT")
            vsh = spool.tile([128, NVT * DV], dt.bfloat16, tag="vsh")
            nc.sync.dma_start(kT[:, 0:CH], kT_d[s, :, 0:CH])
            nc.sync.dma_start(qT[:, 0:CH], qT_d[s, :, 0:CH])
            vn0 = spool.tile([128, DV], dt.bfloat16, tag="vn0")
            nc.sync.dma_start(vn0[:], vn0_d[s])
            vsr = spool.tile([128, DV], dt.bfloat16, tag="vsr")
            nc.sync.dma_start(vsr[:], vsr_d[s])
            if s == 0:
                # group-0 C mask ahead of the rest; big mCg tail goes last
                nc.sync.dma_start(mCg[:, 0:512], mCg_d[:, 0:512])
            nc.sync.dma_start(vsh[:, 0:VCH], vsh_d[s, :, 0:VCH])
            for c0 in range(CH, S, CH):
                nc.sync.dma_start(kT[:, c0:c0 + CH], kT_d[s, :, c0:c0 + CH])
                nc.sync.dma_start(qT[:, c0:c0 + CH], qT_d[s, :, c0:c0 + CH])
                v0 = c0 // CH * VCH
                v1 = min(v0 + VCH, NVT * DV)
                nc.sync.dma_start(vsh[:, v0:v1], vsh_d[s, :, v0:v1])
            if s == 0:
                nc.sync.dma_start(mCg[:, 512:NG * 512], mCg_d[:, 512:NG * 512])
            out_acc = opool.tile([128, NT * D], dt.bfloat16, tag="out_acc")
            cur.update(qT=qT, kT=kT, vsh=vsh, kTsr=kTsr, vsr=vsr, vn0=vn0,
                       out_acc=out_acc)

            def out_chunk(c):
                nc.sync.dma_start(out_d[s, :, OCH * c:OCH * (c + 1)],
                                  out_acc[:, OCH * c:OCH * (c + 1)])

            cgroup(0)
            emit_a(0)
            finish(0)
            emit_a(1)
            for i in range(1, NT):
                if i % GT == GT - 1 and i + 1 < NT:
                    cgroup((i + 1) // GT)
                emit_b(i)
                if i + 1 < NT:
                    emit_a(i + 1)
                finish(i)
                if i >= 2:
                    pv(i - 2)
                    if (i - 2) % 8 == 7:
                        out_chunk((i - 2) // 8)
            pv(NT - 2)
            pv(NT - 1)
            out_chunk(3)

    nc.finalize()
    _prog_cache["nc"] = nc
    return nc


def _prep_core_inputs(q, k, v, rk, rv, consts):
    """q,k,v: [SLICES, S, D] fp32 for one core; rk, rv: [SLICES, R, D]."""
    mAB, mAB0, mCg = consts
    qb = q.astype(BF16)
    kb = k.astype(BF16)
    vb = v.astype(BF16)
    qT = np.ascontiguousarray(qb.transpose(0, 2, 1))          # [SL, 128, S]
    kT = np.ascontiguousarray(kb.transpose(0, 2, 1))
    # 64-shifted padded v tiles augmented with a ones column, stored
    # per-partition-contiguous: [SL, 128, NVT*DV]; tile j = v rows 128j-64..128j+64
    vpad = np.concatenate([np.zeros((SLICES, 64, D), BF16), vb,
                           np.zeros((SLICES, 64, D), BF16)], axis=1)  # [SL, 4224, D]
    vpad = np.concatenate([vpad, np.ones((SLICES, NVT * 128, 1), BF16)], axis=2)
    vsh = np.ascontiguousarray(
        vpad.reshape(SLICES, NVT, 128, DV).transpose(0, 2, 1, 3).reshape(SLICES, 128, NVT * DV))
    # interleaved strided/relay keys, d-major: col 2s = k[64s], col 2s+1 = rk[s]
    ksr_int = np.empty((SLICES, 128, D), BF16)
    ksr_int[:, 0::2] = kb[:, ::W, :]
    ksr_int[:, 1::2] = rk.astype(BF16)
    kTsr = np.ascontiguousarray(ksr_int.transpose(0, 2, 1))           # [SL, 128, 128]
    # interleaved [str0, rel0, str1, rel1, ...] + ones column
    vsr_pairs = np.empty((SLICES, 128, D), BF16)
    vsr_pairs[:, 0::2] = vb[:, ::W, :]
    vsr_pairs[:, 1::2] = rv.astype(BF16)
    vsr = np.ascontiguousarray(
        np.concatenate([vsr_pairs, np.ones((SLICES, 128, 1), BF16)], axis=2))
    vn0 = np.ascontiguousarray(
        np.concatenate([vb[:, 0:128, :], np.ones((SLICES, 128, 1), BF16)], axis=2))
    return {
        "qT": qT, "kT": kT, "vsh": vsh, "kTsr": kTsr, "vsr": vsr, "vn0": vn0,
        "mAB": mAB, "mAB0": mAB0, "mCg": mCg,
    }


def make_in_maps(q, k, v, rk, rv):
    consts = _build_consts()
    qf = q.reshape(B * H, S, D)
    kf = k.reshape(B * H, S, D)
    vf = v.reshape(B * H, S, D)
    rkf = rk.reshape(B * H, R, D)
    rvf = rv.reshape(B * H, R, D)
    in_maps = []
    for c in range(NCORES):
        sl = slice(SLICES * c, SLICES * (c + 1))
        in_maps.append(_prep_core_inputs(qf[sl], kf[sl], vf[sl], rkf[sl], rvf[sl],
                                         consts))
    return in_maps


def kernel(q, k, v, rk, rv, _run_kwargs=None):
    q = np.asarray(q, dtype=np.float32)
    k = np.asarray(k, dtype=np.float32)
    v = np.asarray(v, dtype=np.float32)
    rk = np.asarray(rk, dtype=np.float32)
    rv = np.asarray(rv, dtype=np.float32)
    nc = build_program()
    in_maps = make_in_maps(q, k, v, rk, rv)
    res = run_bass_kernel_spmd(nc, in_maps, list(range(NCORES)), **(_run_kwargs or {}))
    out = np.stack([np.asarray(res.results[c]["out"]) for c in range(NCORES)])
    if _run_kwargs:
        kernel.last_results = res
    # out: [NCORES, SLICES, 128, NT*D] -> [B,H,S,D]
    out = out.reshape(B * H, 128, NT, D).transpose(0, 2, 1, 3)
    return out.reshape(B, H, S, D).astype(np.float32)
